# revision 11
# baseline (speedup 1.0000x reference)
"""AdaptiveGraphConv on 8 TRN2 NeuronCores (Bass/Tile).

Strategy: destination-sharded edge-parallel.  Edges are sorted by dst and
sharded by dst-node range (NLOC nodes/core), so each core owns the full
aggregation for its dst range and NO all-reduce of the [N, D] aggregate is
needed.  Node features h (and A = h@We1_top + be1) are computed node-parallel
and exchanged with a single AllGather of a bf16 [h|A] table; B = h@We1_bot is
only ever indexed by dst (always core-local).  Per-edge work uses dma_gather
(SWDGE) + TensorE matmuls (transpose-accumulate for the edge MLP, one-hot
matmul for the segment scatter-add, PSUM-accumulated per 128-dst group).

dma_gather indices are int16, so the [h|A] table is gathered through two
views: rows [0, 32768) and [32768, NPAD).  Each group's edges are split into
a lo and a hi sub-list, each padded to whole 128-edge tiles (pad edges use
index 0 and colrel=-1, which zeroes their one-hot row).
"""

import math
import os
import sys
import types

sys.path.insert(0, "/opt/trn_rl_repo")

import numpy as np
import ml_dtypes

import concourse.bass as bass
import concourse.bacc as bacc
import concourse.tile as tile
from concourse import mybir
from concourse.bass_utils import run_bass_kernel_spmd

BF16 = ml_dtypes.bfloat16
F32 = mybir.dt.float32
BF = mybir.dt.bfloat16
I16 = mybir.dt.int16

N_CORES = 8
D = 128
P = 128
SPLIT = 32768      # int16 index limit for dma_gather
CHUNK_G = 2        # groups per gather call
SCORE_BATCH = 4    # tiles per relu batch
# debug bisect: 1 = skip gathers+edge compute, 2 = gathers but no edge compute
DBG = int(os.environ.get("GK_DBG", "0"))


def _install_ntff_hook():
    if "antenv.axon_hooks" in sys.modules:
        return
    try:
        from trn_agent_boot.trn_boot import _ntff_profile_via_ctypes

        hook = _ntff_profile_via_ctypes("/opt/axon/libaxon_pjrt.so")
    except Exception:
        hook = None
    mod = types.ModuleType("antenv.axon_hooks")
    mod.get_axon_ntff_profile_hook = lambda: hook
    mod.set_axon_ntff_profile_hook = lambda h: None
    sys.modules["antenv.axon_hooks"] = mod


# ----------------------------------------------------------------------------
# device program
# ----------------------------------------------------------------------------

def _build_program(NG, NLOC, NPAD, Tlo, Thi):
    """Tlo/Thi: per-group tile counts for lo/hi row-gather sub-lists."""
    T_list = [lo + hi for lo, hi in zip(Tlo, Thi)]
    NT = sum(T_list)
    NTlo = sum(Tlo)
    NThi = sum(Thi)
    hi_base = min(SPLIT, NPAD)

    nc = bacc.Bacc(
        "TRN2", target_bir_lowering=False, debug=False, num_devices=N_CORES
    )

    def din(name, shape, dt):
        return nc.dram_tensor(name, list(shape), dt, kind="ExternalInput").ap()

    xT = din("xT", [P, NLOC], F32)
    W1 = din("W1", [P, D], F32)
    b1row = din("b1row", [1, D], F32)
    ones1f = din("ones1f", [1, D], F32)
    ones1b = din("ones1b", [1, D], BF)
    g1b = din("g1b", [P, D], F32)
    bt1b = din("bt1b", [P, D], F32)
    We1T = din("We1T", [P, D], BF)
    We1B = din("We1B", [P, D], BF)
    be1row = din("be1row", [1, D], BF)
    We2c = din("We2c", [P, 1], BF)
    be2c = din("be2c", [P, 1], F32)
    WgT = din("WgT", [P, D], BF)
    WgB = din("WgB", [P, D], BF)
    bgc = din("bgc", [P, 1], F32)
    g2b = din("g2b", [P, D], F32)
    bt2b = din("bt2b", [P, D], F32)
    idn = din("idn", [P, P], BF)
    iota = din("iota", [P, P], BF)
    ixlo = din("ixlo", [P, max(NTlo, 1) * 8], I16)
    ixhi = din("ixhi", [P, max(NThi, 1) * 8], I16)
    ixb = din("ixb", [P, NT * 8], I16)
    colrel = din("colrel", [P, NT], F32)

    out = nc.dram_tensor("out", [NLOC, D], F32, kind="ExternalOutput").ap()

    # chunking of groups for gather calls
    chunks = []
    g0 = 0
    while g0 < NG:
        g1 = min(g0 + CHUNK_G, NG)
        chunks.append((g0, g1))
        g0 = g1
    TloC = max(sum(Tlo[g0:g1]) for g0, g1 in chunks)
    ThiC = max(sum(Thi[g0:g1]) for g0, g1 in chunks)
    TC = max(sum(T_list[g0:g1]) for g0, g1 in chunks)

    with tile.TileContext(nc, trace_sim=False) as tc:
        with (
            tc.tile_pool(name="singles", bufs=1) as sg,
            tc.tile_pool(name="dram", bufs=1, space="DRAM") as dram,
        ):
            def load(ap_in, shape, dt, name, eng=None):
                t = sg.tile(list(shape), dt, name=name)
                nc.sync.dma_start(out=t[:], in_=ap_in[:])
                return t

            W1_sb = load(W1, [P, D], F32, "W1_sb")
            b1_sb = load(b1row, [1, D], F32, "b1_sb")
            o1f_sb = load(ones1f, [1, D], F32, "o1f_sb")
            o1b_sb = load(ones1b, [1, D], BF, "o1b_sb")
            g1_sb = load(g1b, [P, D], F32, "g1_sb")
            bt1_sb = load(bt1b, [P, D], F32, "bt1_sb")
            We1T_sb = load(We1T, [P, D], BF, "We1T_sb")
            We1B_sb = load(We1B, [P, D], BF, "We1B_sb")
            be1_sb = load(be1row, [1, D], BF, "be1_sb")
            We2_sb = load(We2c, [P, 1], BF, "We2_sb")
            be2_sb = load(be2c, [P, 1], F32, "be2_sb")
            WgT_sb = load(WgT, [P, D], BF, "WgT_sb")
            WgB_sb = load(WgB, [P, D], BF, "WgB_sb")
            bg_sb = load(bgc, [P, 1], F32, "bg_sb")
            g2_sb = load(g2b, [P, D], F32, "g2_sb")
            bt2_sb = load(bt2b, [P, D], F32, "bt2_sb")
            idn_sb = load(idn, [P, P], BF, "idn_sb")
            iota_sb = load(iota, [P, P], BF, "iota_sb")
            ixlo_sb = load(ixlo, [P, max(NTlo, 1) * 8], I16, "ixlo_sb")
            ixhi_sb = load(ixhi, [P, max(NThi, 1) * 8], I16, "ixhi_sb")
            ixb_sb = load(ixb, [P, NT * 8], I16, "ixb_sb")
            crel_sb = load(colrel, [P, NT], F32, "crel_sb")

            eps_sb = sg.tile([P, 1], F32, name="eps_sb")
            nc.vector.memset(eps_sb[:], 1e-5)

            h_sb = sg.tile([P, NLOC], BF, name="h_sb")
            hT_sb = sg.tile([P, NLOC], BF, name="hT_sb")
            A_sb = sg.tile([P, NLOC], BF, name="A_sb")
            B_sb = sg.tile([P, NLOC], BF, name="B_sb")

            HA_shard = dram.tile([NLOC, 2 * D], BF, name="HA_shard")
            HA_full = dram.tile(
                [NPAD, 2 * D], BF, name="HA_full", addr_space="Shared"
            )
            B_dram = dram.tile([NLOC, D], BF, name="B_dram")

            # ================= phase 1: node transform ==================
            with (
                tc.tile_pool(name="xtp", bufs=1) as xtp,
                tc.tile_pool(name="ps1", bufs=2, space="PSUM") as ps1,
                tc.tile_pool(name="w1p", bufs=3) as w1p,
            ):
                xT_sb = xtp.tile([P, NLOC], F32, name="xT_sb")
                nc.sync.dma_start(out=xT_sb[:], in_=xT[:])
                for g in range(NG):
                    gsl = slice(g * P, (g + 1) * P)
                    hp = ps1.tile([P, D], F32, tag="hpre", name=f"hp{g}")
                    nc.tensor.matmul(
                        out=hp[:], lhsT=xT_sb[:, gsl], rhs=W1_sb[:],
                        start=True, stop=False,
                    )
                    nc.tensor.matmul(
                        out=hp[:], lhsT=o1f_sb[:], rhs=b1_sb[:],
                        start=False, stop=True,
                    )
                    st = w1p.tile([P, 6], F32, tag="st", name=f"st{g}")
                    nc.vector.bn_stats(out=st[:], in_=hp[:])
                    mv = w1p.tile([P, 2], F32, tag="mv", name=f"mv{g}")
                    nc.vector.bn_aggr(out=mv[:], in_=st[:])
                    sd = w1p.tile([P, 1], F32, tag="sd", name=f"sd{g}")
                    nc.scalar.activation(
                        out=sd[:], in_=mv[:, 1:2],
                        func=mybir.ActivationFunctionType.Sqrt,
                        bias=eps_sb[:],
                    )
                    rstd = w1p.tile([P, 1], F32, tag="rstd", name=f"rs{g}")
                    nc.vector.reciprocal(out=rstd[:], in_=sd[:])
                    t1 = w1p.tile([P, D], F32, tag="t1", name=f"t1{g}")
                    nc.vector.tensor_scalar(
                        out=t1[:], in0=hp[:], scalar1=mv[:, 0:1],
                        scalar2=rstd[:], op0=mybir.AluOpType.subtract,
                        op1=mybir.AluOpType.mult,
                    )
                    u1 = w1p.tile([P, D], F32, tag="u1", name=f"u1{g}")
                    nc.vector.tensor_mul(out=u1[:], in0=t1[:], in1=g1_sb[:])
                    v1 = w1p.tile([P, D], F32, tag="v1", name=f"v1{g}")
                    nc.vector.tensor_add(out=v1[:], in0=u1[:], in1=bt1_sb[:])
                    nc.vector.tensor_scalar_max(
                        out=h_sb[:, gsl], in0=v1[:], scalar1=0.0
                    )
                    htp = ps1.tile([P, D], F32, tag="hT", name=f"htp{g}")
                    nc.tensor.matmul(
                        out=htp[:], lhsT=h_sb[:, gsl], rhs=idn_sb[:],
                        start=True, stop=True,
                    )
                    nc.any.tensor_copy(out=hT_sb[:, gsl], in_=htp[:])
                    ap_ = ps1.tile([P, D], F32, tag="A", name=f"apz{g}")
                    nc.tensor.matmul(
                        out=ap_[:], lhsT=hT_sb[:, gsl], rhs=We1T_sb[:],
                        start=True, stop=False,
                    )
                    nc.tensor.matmul(
                        out=ap_[:], lhsT=o1b_sb[:], rhs=be1_sb[:],
                        start=False, stop=True,
                    )
                    nc.any.tensor_copy(out=A_sb[:, gsl], in_=ap_[:])
                    bp = ps1.tile([P, D], F32, tag="B", name=f"bp{g}")
                    nc.tensor.matmul(
                        out=bp[:], lhsT=hT_sb[:, gsl], rhs=We1B_sb[:],
                        start=True, stop=True,
                    )
                    nc.any.tensor_copy(out=B_sb[:, gsl], in_=bp[:])

            ha_v = HA_shard.rearrange("(g p) c -> p g c", p=P)
            nc.sync.dma_start(
                out=ha_v[:, :, 0:D],
                in_=h_sb.rearrange("p (g j) -> p g j", g=NG),
            )
            nc.sync.dma_start(
                out=ha_v[:, :, D : 2 * D],
                in_=A_sb.rearrange("p (g j) -> p g j", g=NG),
            )
            nc.sync.dma_start(
                out=B_dram.rearrange("(g p) c -> p g c", p=P)[:, :, :],
                in_=B_sb.rearrange("p (g j) -> p g j", g=NG),
            )
            nc.gpsimd.collective_compute(
                "AllGather",
                mybir.AluOpType.bypass,
                replica_groups=[list(range(N_CORES))],
                ins=[HA_shard.opt()],
                outs=[HA_full.opt()],
            )

            # ================= phase 2+3: edges + update ================
            with (
                tc.tile_pool(name="pz", bufs=2, space="PSUM") as pz,
                tc.tile_pool(name="psc", bufs=2, space="PSUM") as psc,
                tc.tile_pool(name="pag", bufs=2, space="PSUM") as pag,
                tc.tile_pool(name="pg3", bufs=1, space="PSUM") as pg3,
                tc.tile_pool(name="gio", bufs=2) as gio,
                tc.tile_pool(name="wrk", bufs=3) as wrk,
                tc.tile_pool(name="osb", bufs=2) as osb,
            ):
                lo_off = 0   # tile offsets into the lo / hi / combined seqs
                hi_off = 0
                t_off = 0
                for (ga, gb) in chunks:
                    nlo = sum(Tlo[ga:gb])
                    nhi = sum(Thi[ga:gb])
                    ntc = sum(T_list[ga:gb])
                    halo = gio.tile(
                        [P, max(TloC, 1), 2 * D], BF, tag="halo",
                        name=f"halo{ga}",
                    )
                    hahi = gio.tile(
                        [P, max(ThiC, 1), 2 * D], BF, tag="hahi",
                        name=f"hahi{ga}",
                    )
                    btc = gio.tile([P, TC, D], BF, tag="btc", name=f"btc{ga}")
                    if nlo and DBG != 1:
                        nc.gpsimd.dma_gather(
                            out_ap=halo[:, 0:nlo, :],
                            in_ap=HA_full[0:hi_base, :],
                            idxs_ap=ixlo_sb[:, lo_off * 8 : (lo_off + nlo) * 8],
                            num_idxs=nlo * P,
                            num_idxs_reg=nlo * P,
                            elem_size=2 * D,
                            single_packet=False,
                        )
                    if nhi and DBG != 1:
                        nc.gpsimd.dma_gather(
                            out_ap=hahi[:, 0:nhi, :],
                            in_ap=HA_full[hi_base:, :],
                            idxs_ap=ixhi_sb[:, hi_off * 8 : (hi_off + nhi) * 8],
                            num_idxs=nhi * P,
                            num_idxs_reg=nhi * P,
                            elem_size=2 * D,
                            single_packet=False,
                        )
                    if DBG != 1:
                        nc.gpsimd.dma_gather(
                            out_ap=btc[:, 0:ntc, :],
                            in_ap=B_dram[:, :],
                            idxs_ap=ixb_sb[:, t_off * 8 : (t_off + ntc) * 8],
                            num_idxs=ntc * P,
                            num_idxs_reg=ntc * P,
                            elem_size=D,
                            single_packet=False,
                        )
                    lpos = 0
                    hpos = 0
                    tpos = 0
                    for g in range(ga, gb):
                        Tg = T_list[g]
                        gsl = slice(g * P, (g + 1) * P)

                        def ha_t(j, _l=lpos, _h=hpos, _g=g):
                            if j < Tlo[_g]:
                                return halo[:, _l + j, :]
                            return hahi[:, _h + (j - Tlo[_g]), :]

                        s_ps = psc.tile([P, Tg], F32, tag="s", name=f"s{g}")
                        aggr = pag.tile([P, P], F32, tag="aggr", name=f"ag{g}")
                        w_sb = wrk.tile([P, Tg], F32, tag="w", name=f"w{g}")
                        nck = (Tg + SCORE_BATCH - 1) // SCORE_BATCH
                        if DBG:
                            nck = 0
                        for c in range(nck):
                            tl, th = c * SCORE_BATCH, min((c + 1) * SCORE_BATCH, Tg)
                            z = pz.tile(
                                [P, SCORE_BATCH * P], F32, tag="z",
                                name=f"z{g}_{c}",
                            )
                            for i, t in enumerate(range(tl, th)):
                                zsl = slice(i * P, (i + 1) * P)
                                nc.tensor.matmul(
                                    out=z[:, zsl], lhsT=ha_t(t)[:, D : 2 * D],
                                    rhs=idn_sb[:], start=True, stop=False,
                                )
                                nc.tensor.matmul(
                                    out=z[:, zsl], lhsT=btc[:, tpos + t, :],
                                    rhs=idn_sb[:], start=False, stop=True,
                                )
                            wl = (th - tl) * P
                            r = wrk.tile(
                                [P, SCORE_BATCH * P], BF, tag="r",
                                name=f"r{g}_{c}",
                            )
                            nc.scalar.activation(
                                out=r[:, 0:wl], in_=z[:, 0:wl],
                                func=mybir.ActivationFunctionType.Relu,
                            )
                            for i, t in enumerate(range(tl, th)):
                                nc.tensor.matmul(
                                    out=s_ps[:, t : t + 1],
                                    lhsT=r[:, i * P : (i + 1) * P],
                                    rhs=We2_sb[:], start=True, stop=True,
                                )
                        if not DBG:
                            nc.scalar.activation(
                                out=w_sb[:], in_=s_ps[:, 0:Tg],
                                func=mybir.ActivationFunctionType.Sigmoid,
                                bias=be2_sb[:],
                            )
                        for t in range(Tg if not DBG else 0):
                            m = wrk.tile([P, P], BF, tag="m", name=f"m{g}_{t}")
                            nc.vector.tensor_scalar(
                                out=m[:], in0=iota_sb[:],
                                scalar1=crel_sb[:, t_off + tpos + t : t_off + tpos + t + 1],
                                scalar2=w_sb[:, t : t + 1],
                                op0=mybir.AluOpType.is_equal,
                                op1=mybir.AluOpType.mult,
                            )
                            nc.tensor.matmul(
                                out=aggr[:], lhsT=ha_t(t)[:, 0:D], rhs=m[:],
                                start=(t == 0), stop=(t == Tg - 1),
                            )
                        # ---- phase 3 ----
                        ragg = wrk.tile([P, P], BF, tag="ragg", name=f"rg{g}")
                        if DBG:
                            nc.vector.memset(ragg[:], 0.0)
                        else:
                            nc.any.tensor_copy(out=ragg[:], in_=aggr[:])
                        gp = pg3.tile([P, P], F32, tag="gate", name=f"gp{g}")
                        nc.tensor.matmul(
                            out=gp[:], lhsT=WgT_sb[:], rhs=hT_sb[:, gsl],
                            start=True, stop=False,
                        )
                        nc.tensor.matmul(
                            out=gp[:], lhsT=WgB_sb[:], rhs=ragg[:],
                            start=False, stop=True,
                        )
                        gate = wrk.tile([P, P], BF, tag="gate_sb", name=f"gt{g}")
                        nc.scalar.activation(
                            out=gate[:], in_=gp[:],
                            func=mybir.ActivationFunctionType.Sigmoid,
                            bias=bg_sb[:],
                        )
                        d1 = wrk.tile([P, P], BF, tag="d1", name=f"d1{g}")
                        nc.vector.tensor_tensor(
                            out=d1[:], in0=ragg[:], in1=hT_sb[:, gsl],
                            op=mybir.AluOpType.subtract,
                        )
                        d2 = wrk.tile([P, P], BF, tag="d2", name=f"d2{g}")
                        nc.vector.tensor_mul(out=d2[:], in0=gate[:], in1=d1[:])
                        hn = wrk.tile([P, P], BF, tag="hn", name=f"hn{g}")
                        nc.vector.tensor_add(
                            out=hn[:], in0=hT_sb[:, gsl], in1=d2[:]
                        )
                        hnp = pg3.tile([P, P], F32, tag="hnp", name=f"hnp{g}")
                        nc.tensor.matmul(
                            out=hnp[:], lhsT=hn[:], rhs=idn_sb[:],
                            start=True, stop=True,
                        )
                        st3 = wrk.tile([P, 6], F32, tag="st3", name=f"st3{g}")
                        nc.vector.bn_stats(out=st3[:], in_=hnp[:])
                        mv3 = wrk.tile([P, 2], F32, tag="mv3", name=f"mv3{g}")
                        nc.vector.bn_aggr(out=mv3[:], in_=st3[:])
                        sd3 = wrk.tile([P, 1], F32, tag="sd3", name=f"sd3{g}")
                        nc.scalar.activation(
                            out=sd3[:], in_=mv3[:, 1:2],
                            func=mybir.ActivationFunctionType.Sqrt,
                            bias=eps_sb[:],
                        )
                        rstd3 = wrk.tile([P, 1], F32, tag="rstd3", name=f"rs3{g}")
                        nc.vector.reciprocal(out=rstd3[:], in_=sd3[:])
                        t1o = osb.tile([P, D], F32, tag="t1o", name=f"t1o{g}")
                        nc.vector.tensor_scalar(
                            out=t1o[:], in0=hnp[:], scalar1=mv3[:, 0:1],
                            scalar2=rstd3[:], op0=mybir.AluOpType.subtract,
                            op1=mybir.AluOpType.mult,
                        )
                        u1o = osb.tile([P, D], F32, tag="u1o", name=f"u1o{g}")
                        nc.vector.tensor_mul(out=u1o[:], in0=t1o[:], in1=g2_sb[:])
                        o1o = osb.tile([P, D], F32, tag="o1o", name=f"o1o{g}")
                        nc.vector.tensor_add(out=o1o[:], in0=u1o[:], in1=bt2_sb[:])
                        nc.sync.dma_start(out=out[gsl, :], in_=o1o[:])
                        lpos += Tlo[g]
                        hpos += Thi[g]
                        tpos += Tg
                    lo_off += nlo
                    hi_off += nhi
                    t_off += ntc

    nc.compile()
    return nc


# ----------------------------------------------------------------------------
# host-side sharding + launch
# ----------------------------------------------------------------------------

_CACHE = {}


def _wrap16(seq):
    """idx i -> [i%16, i//16], replicated to 128 partitions (8 Q7 cores)."""
    n = len(seq)
    assert n % 16 == 0
    blk = np.asarray(seq, np.int16).reshape(-1, 16).T  # [16, n/16]
    return np.tile(blk, (8, 1))                        # [128, n/16]


def kernel(
    x, edge_index, W1, b1, g1, bt1, We1, be1, We2, be2,
    Wn1, bn1, Wn2, bn2, Wg, bg, g2, bt2, _trace=False,
):
    x = np.asarray(x, dtype=np.float32)
    N = x.shape[0]
    NG = (N + N_CORES * P - 1) // (N_CORES * P)
    NLOC = NG * P
    NPAD = NLOC * N_CORES

    row = np.asarray(edge_index[0], dtype=np.int64)
    col = np.asarray(edge_index[1], dtype=np.int64)
    order = np.argsort(col, kind="stable")
    row_s = row[order].astype(np.int32)
    col_s = col[order].astype(np.int32)

    bounds = np.searchsorted(col_s, np.arange(N_CORES + 1) * NLOC)
    cnt_lo = np.zeros((N_CORES, NG), dtype=np.int64)
    cnt_hi = np.zeros((N_CORES, NG), dtype=np.int64)
    for k in range(N_CORES):
        lo, hi = bounds[k], bounds[k + 1]
        gloc = (col_s[lo:hi] - k * NLOC) // P
        is_hi = row_s[lo:hi] >= SPLIT
        cnt_lo[k] = np.bincount(gloc[~is_hi], minlength=NG)
        cnt_hi[k] = np.bincount(gloc[is_hi], minlength=NG)
    Tlo = [int(math.ceil(cnt_lo[:, g].max() / P)) for g in range(NG)]
    Thi = [int(math.ceil(cnt_hi[:, g].max() / P)) for g in range(NG)]
    for g in range(NG):
        if Tlo[g] + Thi[g] == 0:
            Tlo[g] = 1
    T_list = [lo + hi for lo, hi in zip(Tlo, Thi)]
    NT = sum(T_list)

    key = (N, NG, tuple(Tlo), tuple(Thi))
    if key not in _CACHE:
        _CACHE[key] = _build_program(NG, NLOC, NPAD, Tlo, Thi)
    nc = _CACHE[key]

    bf = lambda a: np.ascontiguousarray(np.asarray(a, np.float32)).astype(BF16)
    f32 = lambda a: np.ascontiguousarray(np.asarray(a, np.float32))
    shared = {
        "W1": f32(W1),
        "b1row": f32(b1).reshape(1, D),
        "ones1f": np.ones((1, D), np.float32),
        "ones1b": np.ones((1, D), BF16),
        "g1b": np.broadcast_to(f32(g1).reshape(1, D), (P, D)).copy(),
        "bt1b": np.broadcast_to(f32(bt1).reshape(1, D), (P, D)).copy(),
        "We1T": bf(We1[:D]),
        "We1B": bf(We1[D:]),
        "be1row": bf(be1).reshape(1, D),
        "We2c": bf(We2).reshape(P, 1),
        "be2c": np.broadcast_to(f32(be2).reshape(1, 1), (P, 1)).copy(),
        "WgT": bf(Wg[:D]),
        "WgB": bf(Wg[D:]),
        "bgc": f32(bg).reshape(P, 1),
        "g2b": np.broadcast_to(f32(g2).reshape(1, D), (P, D)).copy(),
        "bt2b": np.broadcast_to(f32(bt2).reshape(1, D), (P, D)).copy(),
        "idn": np.eye(P, dtype=BF16),
        "iota": np.broadcast_to(
            np.arange(P, dtype=np.float32).reshape(1, P), (P, P)
        ).astype(BF16),
    }

    xp = np.zeros((NPAD, D), np.float32)
    xp[:N] = x

    in_maps = []
    for k in range(N_CORES):
        lo, hi = bounds[k], bounds[k + 1]
        rk = row_s[lo:hi]
        ck = col_s[lo:hi] - k * NLOC
        gk = ck // P
        is_hi = rk >= SPLIT
        seq_lo = []     # row idx (lo), per group padded
        seq_hi = []     # row idx - SPLIT (hi)
        seq_b = []      # col idx, combined tile order
        seq_cr = []     # col - group_base (or -1), combined tile order
        for g in range(NG):
            sel = gk == g
            for half, Tn, base in ((~is_hi & sel, Tlo[g], 0),
                                   (is_hi & sel, Thi[g], SPLIT)):
                n = int(half.sum())
                npad = Tn * P
                r_g = np.zeros(npad, np.int32)
                c_g = np.zeros(npad, np.int32)
                x_g = np.full(npad, -1.0, np.float32)
                r_g[:n] = rk[half] - base
                c_g[:n] = ck[half]
                x_g[:n] = (c_g[:n] - g * P).astype(np.float32)
                (seq_lo if base == 0 else seq_hi).append(r_g)
                seq_b.append(c_g)
                seq_cr.append(x_g)
        cat = lambda s, n: (np.concatenate(s) if s else np.zeros(0, np.int32))
        slo = np.concatenate(seq_lo) if seq_lo else np.zeros(0, np.int32)
        shi = np.concatenate(seq_hi) if seq_hi else np.zeros(0, np.int32)
        sb = np.concatenate(seq_b)
        scr = np.concatenate(seq_cr)
        if len(slo) == 0:
            slo = np.zeros(P, np.int32)
        if len(shi) == 0:
            shi = np.zeros(P, np.int32)
        im = dict(shared)
        im["xT"] = np.ascontiguousarray(xp[k * NLOC : (k + 1) * NLOC].T)
        im["ixlo"] = _wrap16(slo)
        im["ixhi"] = _wrap16(shi)
        im["ixb"] = _wrap16(sb)
        im["colrel"] = np.ascontiguousarray(
            scr.reshape(NT, P).T
        )
        in_maps.append(im)

    if _trace:
        _install_ntff_hook()
    res = run_bass_kernel_spmd(
        nc, in_maps, core_ids=list(range(N_CORES)), trace=_trace
    )
    out = np.concatenate(
        [res.results[k]["out"] for k in range(N_CORES)], axis=0
    )[:N]
    if _trace:
        kernel.last_exec_time_ns = res.exec_time_ns
    return np.ascontiguousarray(out, dtype=np.float32)


# revision 13
# speedup vs baseline: 1.3967x; 1.3967x over previous
"""AdaptiveGraphConv on 8 TRN2 NeuronCores (Bass/Tile).

Strategy: destination-sharded edge-parallel.  Edges are sorted by dst and
sharded by dst-node range (NLOC nodes/core), so each core owns the full
aggregation for its dst range and NO all-reduce of the [N, D] aggregate is
needed.  Node features h (and A = h@We1_top + be1) are computed node-parallel
and exchanged with a single AllGather of a bf16 [h|A] table; B = h@We1_bot is
only ever indexed by dst (always core-local).  Per-edge work uses dma_gather
(SWDGE) + TensorE matmuls (transpose-accumulate for the edge MLP, one-hot
matmul for the segment scatter-add, PSUM-accumulated per 128-dst group).

dma_gather indices are int16, so the [h|A] table is gathered through two
views: rows [0, 32768) and [32768, NPAD).  Each group's edges are split into
a lo and a hi sub-list, each padded to whole 128-edge tiles (pad edges use
index 0 and colrel=-1, which zeroes their one-hot row).
"""

import math
import os
import sys
import types

sys.path.insert(0, "/opt/trn_rl_repo")

import numpy as np
import ml_dtypes

import concourse.bass as bass
import concourse.bacc as bacc
import concourse.tile as tile
from concourse import mybir
from concourse.bass_utils import run_bass_kernel_spmd

BF16 = ml_dtypes.bfloat16
F32 = mybir.dt.float32
BF = mybir.dt.bfloat16
I16 = mybir.dt.int16

N_CORES = 8
D = 128
P = 128
SPLIT = 32768      # int16 index limit for dma_gather
CHUNK_G = 4        # groups per gather call
SCORE_BATCH = 4    # tiles per relu batch
# debug bisect: 1 = skip gathers+edge compute, 2 = gathers but no edge compute
DBG = int(os.environ.get("GK_DBG", "0"))


def _install_ntff_hook():
    if "antenv.axon_hooks" in sys.modules:
        return
    try:
        from trn_agent_boot.trn_boot import _ntff_profile_via_ctypes

        hook = _ntff_profile_via_ctypes("/opt/axon/libaxon_pjrt.so")
    except Exception:
        hook = None
    mod = types.ModuleType("antenv.axon_hooks")
    mod.get_axon_ntff_profile_hook = lambda: hook
    mod.set_axon_ntff_profile_hook = lambda h: None
    sys.modules["antenv.axon_hooks"] = mod


# ----------------------------------------------------------------------------
# device program
# ----------------------------------------------------------------------------

def _build_program(NG, NLOC, NPAD, Tlo, Thi):
    """Tlo/Thi: per-group tile counts for lo/hi row-gather sub-lists."""
    T_list = [lo + hi for lo, hi in zip(Tlo, Thi)]
    NT = sum(T_list)
    NTlo = sum(Tlo)
    NThi = sum(Thi)
    hi_base = min(SPLIT, NPAD)

    nc = bacc.Bacc(
        "TRN2", target_bir_lowering=False, debug=False, num_devices=N_CORES,
        num_swdge_queues=4,
    )

    def din(name, shape, dt):
        return nc.dram_tensor(name, list(shape), dt, kind="ExternalInput").ap()

    xT = din("xT", [P, NLOC], F32)
    W1 = din("W1", [P, D], F32)
    b1row = din("b1row", [1, D], F32)
    ones1f = din("ones1f", [1, D], F32)
    ones1b = din("ones1b", [1, D], BF)
    g1b = din("g1b", [P, D], F32)
    bt1b = din("bt1b", [P, D], F32)
    We1T = din("We1T", [P, D], BF)
    We1B = din("We1B", [P, D], BF)
    be1row = din("be1row", [1, D], BF)
    We2c = din("We2c", [P, 1], BF)
    be2c = din("be2c", [P, 1], F32)
    WgT = din("WgT", [P, D], BF)
    WgB = din("WgB", [P, D], BF)
    bgc = din("bgc", [P, 1], F32)
    g2b = din("g2b", [P, D], F32)
    bt2b = din("bt2b", [P, D], F32)
    idn = din("idn", [P, P], BF)
    iota = din("iota", [P, P], BF)
    ixlo = din("ixlo", [P, max(NTlo, 1) * 8], I16)
    ixhi = din("ixhi", [P, max(NThi, 1) * 8], I16)
    ixb = din("ixb", [P, NT * 8], I16)
    colrel = din("colrel", [P, NT], F32)

    out = nc.dram_tensor("out", [NLOC, D], F32, kind="ExternalOutput").ap()

    # chunking of groups for gather calls
    chunks = []
    g0 = 0
    while g0 < NG:
        g1 = min(g0 + CHUNK_G, NG)
        chunks.append((g0, g1))
        g0 = g1
    TloC = max(sum(Tlo[g0:g1]) for g0, g1 in chunks)
    ThiC = max(sum(Thi[g0:g1]) for g0, g1 in chunks)
    TC = max(sum(T_list[g0:g1]) for g0, g1 in chunks)

    with tile.TileContext(nc, trace_sim=False) as tc:
        with (
            tc.tile_pool(name="singles", bufs=1) as sg,
            tc.tile_pool(name="dram", bufs=1, space="DRAM") as dram,
        ):
            def load(ap_in, shape, dt, name, eng=None):
                t = sg.tile(list(shape), dt, name=name)
                nc.sync.dma_start(out=t[:], in_=ap_in[:])
                return t

            W1_sb = load(W1, [P, D], F32, "W1_sb")
            b1_sb = load(b1row, [1, D], F32, "b1_sb")
            o1f_sb = load(ones1f, [1, D], F32, "o1f_sb")
            o1b_sb = load(ones1b, [1, D], BF, "o1b_sb")
            g1_sb = load(g1b, [P, D], F32, "g1_sb")
            bt1_sb = load(bt1b, [P, D], F32, "bt1_sb")
            We1T_sb = load(We1T, [P, D], BF, "We1T_sb")
            We1B_sb = load(We1B, [P, D], BF, "We1B_sb")
            be1_sb = load(be1row, [1, D], BF, "be1_sb")
            We2_sb = load(We2c, [P, 1], BF, "We2_sb")
            be2_sb = load(be2c, [P, 1], F32, "be2_sb")
            WgT_sb = load(WgT, [P, D], BF, "WgT_sb")
            WgB_sb = load(WgB, [P, D], BF, "WgB_sb")
            bg_sb = load(bgc, [P, 1], F32, "bg_sb")
            g2_sb = load(g2b, [P, D], F32, "g2_sb")
            bt2_sb = load(bt2b, [P, D], F32, "bt2_sb")
            idn_sb = load(idn, [P, P], BF, "idn_sb")
            iota_sb = load(iota, [P, P], BF, "iota_sb")
            ixlo_sb = load(ixlo, [P, max(NTlo, 1) * 8], I16, "ixlo_sb")
            ixhi_sb = load(ixhi, [P, max(NThi, 1) * 8], I16, "ixhi_sb")
            ixb_sb = load(ixb, [P, NT * 8], I16, "ixb_sb")
            crel_sb = load(colrel, [P, NT], F32, "crel_sb")

            eps_sb = sg.tile([P, 1], F32, name="eps_sb")
            nc.vector.memset(eps_sb[:], 1e-5)

            hT_sb = sg.tile([P, NLOC], BF, name="hT_sb")
            hn_sb = sg.tile([P, NLOC], BF, name="hn_sb")

            HA_shard = dram.tile([NLOC, 2 * D], BF, name="HA_shard")
            HA_full = dram.tile(
                [NPAD, 2 * D], BF, name="HA_full", addr_space="Shared"
            )
            B_dram = dram.tile([NLOC, D], BF, name="B_dram")

            # ================= phase 1: node transform ==================
            with (
                tc.tile_pool(name="xtp", bufs=1) as xtp,
                tc.tile_pool(name="ps1", bufs=2, space="PSUM") as ps1,
                tc.tile_pool(name="w1p", bufs=3) as w1p,
            ):
                xT_sb = xtp.tile([P, NLOC], F32, name="xT_sb")
                nc.sync.dma_start(out=xT_sb[:], in_=xT[:])
                h_sb = xtp.tile([P, NLOC], BF, name="h_sb")
                A_sb = xtp.tile([P, NLOC], BF, name="A_sb")
                B_sb = xtp.tile([P, NLOC], BF, name="B_sb")
                for g in range(NG):
                    gsl = slice(g * P, (g + 1) * P)
                    hp = ps1.tile([P, D], F32, tag="hpre", name=f"hp{g}")
                    nc.tensor.matmul(
                        out=hp[:], lhsT=xT_sb[:, gsl], rhs=W1_sb[:],
                        start=True, stop=False,
                    )
                    nc.tensor.matmul(
                        out=hp[:], lhsT=o1f_sb[:], rhs=b1_sb[:],
                        start=False, stop=True,
                    )
                    st = w1p.tile([P, 6], F32, tag="st", name=f"st{g}")
                    nc.vector.bn_stats(out=st[:], in_=hp[:])
                    mv = w1p.tile([P, 2], F32, tag="mv", name=f"mv{g}")
                    nc.vector.bn_aggr(out=mv[:], in_=st[:])
                    sd = w1p.tile([P, 1], F32, tag="sd", name=f"sd{g}")
                    nc.scalar.activation(
                        out=sd[:], in_=mv[:, 1:2],
                        func=mybir.ActivationFunctionType.Sqrt,
                        bias=eps_sb[:],
                    )
                    rstd = w1p.tile([P, 1], F32, tag="rstd", name=f"rs{g}")
                    nc.vector.reciprocal(out=rstd[:], in_=sd[:])
                    t1 = w1p.tile([P, D], F32, tag="t1", name=f"t1{g}")
                    nc.vector.tensor_scalar(
                        out=t1[:], in0=hp[:], scalar1=mv[:, 0:1],
                        scalar2=rstd[:], op0=mybir.AluOpType.subtract,
                        op1=mybir.AluOpType.mult,
                    )
                    u1 = w1p.tile([P, D], F32, tag="u1", name=f"u1{g}")
                    nc.vector.tensor_mul(out=u1[:], in0=t1[:], in1=g1_sb[:])
                    v1 = w1p.tile([P, D], F32, tag="v1", name=f"v1{g}")
                    nc.vector.tensor_add(out=v1[:], in0=u1[:], in1=bt1_sb[:])
                    nc.vector.tensor_scalar_max(
                        out=h_sb[:, gsl], in0=v1[:], scalar1=0.0
                    )
                    htp = ps1.tile([P, D], F32, tag="hT", name=f"htp{g}")
                    nc.tensor.matmul(
                        out=htp[:], lhsT=h_sb[:, gsl], rhs=idn_sb[:],
                        start=True, stop=True,
                    )
                    nc.any.tensor_copy(out=hT_sb[:, gsl], in_=htp[:])
                    ap_ = ps1.tile([P, D], F32, tag="A", name=f"apz{g}")
                    nc.tensor.matmul(
                        out=ap_[:], lhsT=hT_sb[:, gsl], rhs=We1T_sb[:],
                        start=True, stop=False,
                    )
                    nc.tensor.matmul(
                        out=ap_[:], lhsT=o1b_sb[:], rhs=be1_sb[:],
                        start=False, stop=True,
                    )
                    nc.any.tensor_copy(out=A_sb[:, gsl], in_=ap_[:])
                    bp = ps1.tile([P, D], F32, tag="B", name=f"bp{g}")
                    nc.tensor.matmul(
                        out=bp[:], lhsT=hT_sb[:, gsl], rhs=We1B_sb[:],
                        start=True, stop=True,
                    )
                    nc.any.tensor_copy(out=B_sb[:, gsl], in_=bp[:])

                ha_v = HA_shard.rearrange("(g p) c -> p g c", p=P)
                nc.sync.dma_start(
                    out=ha_v[:, :, 0:D],
                    in_=h_sb.rearrange("p (g j) -> p g j", g=NG),
                )
                nc.sync.dma_start(
                    out=ha_v[:, :, D : 2 * D],
                    in_=A_sb.rearrange("p (g j) -> p g j", g=NG),
                )
                nc.sync.dma_start(
                    out=B_dram.rearrange("(g p) c -> p g c", p=P)[:, :, :],
                    in_=B_sb.rearrange("p (g j) -> p g j", g=NG),
                )
            nc.gpsimd.collective_compute(
                "AllGather",
                mybir.AluOpType.bypass,
                replica_groups=[list(range(N_CORES))],
                ins=[HA_shard.opt()],
                outs=[HA_full.opt()],
            )

            # ================= phase 2+3: edges + update ================
            with (
                tc.tile_pool(name="pz", bufs=2, space="PSUM") as pz,
                tc.tile_pool(name="psc", bufs=2, space="PSUM") as psc,
                tc.tile_pool(name="pag", bufs=2, space="PSUM") as pag,
                tc.tile_pool(name="pg3", bufs=1, space="PSUM") as pg3,
                tc.tile_pool(name="gio", bufs=2) as gio,
                tc.tile_pool(name="wrk", bufs=3) as wrk,
                tc.tile_pool(name="osb", bufs=2) as osb,
            ):
                lo_off = 0   # tile offsets into the lo / hi / combined seqs
                hi_off = 0
                t_off = 0
                qctr = [0]

                def next_q():
                    q = qctr[0] % 4
                    qctr[0] += 1
                    return q
                for (ga, gb) in chunks:
                    nlo = sum(Tlo[ga:gb])
                    nhi = sum(Thi[ga:gb])
                    ntc = sum(T_list[ga:gb])
                    halo = gio.tile(
                        [P, max(TloC, 1), 2 * D], BF, tag="halo",
                        name=f"halo{ga}",
                    )
                    hahi = gio.tile(
                        [P, max(ThiC, 1), 2 * D], BF, tag="hahi",
                        name=f"hahi{ga}",
                    )
                    btc = gio.tile([P, TC, D], BF, tag="btc", name=f"btc{ga}")
                    if nlo and DBG != 1:
                        nc.gpsimd.dma_gather(
                            out_ap=halo[:, 0:nlo, :],
                            in_ap=HA_full[0:hi_base, :],
                            idxs_ap=ixlo_sb[:, lo_off * 8 : (lo_off + nlo) * 8],
                            num_idxs=nlo * P,
                            num_idxs_reg=nlo * P,
                            elem_size=2 * D,
                            single_packet=False,
                            queue_num=next_q(),
                        )
                    if nhi and DBG != 1:
                        nc.gpsimd.dma_gather(
                            out_ap=hahi[:, 0:nhi, :],
                            in_ap=HA_full[hi_base:, :],
                            idxs_ap=ixhi_sb[:, hi_off * 8 : (hi_off + nhi) * 8],
                            num_idxs=nhi * P,
                            num_idxs_reg=nhi * P,
                            elem_size=2 * D,
                            single_packet=False,
                            queue_num=next_q(),
                        )
                    if DBG != 1:
                        nb1 = ntc // 2
                        for b0, b1 in ((0, nb1), (nb1, ntc)):
                            if b1 > b0:
                                nc.gpsimd.dma_gather(
                                    out_ap=btc[:, b0:b1, :],
                                    in_ap=B_dram[:, :],
                                    idxs_ap=ixb_sb[
                                        :, (t_off + b0) * 8 : (t_off + b1) * 8
                                    ],
                                    num_idxs=(b1 - b0) * P,
                                    num_idxs_reg=(b1 - b0) * P,
                                    elem_size=D,
                                    single_packet=False,
                                    queue_num=next_q(),
                                )
                    lpos = 0
                    hpos = 0
                    tpos = 0
                    for g in range(ga, gb):
                        Tg = T_list[g]
                        gsl = slice(g * P, (g + 1) * P)

                        def ha_t(j, _l=lpos, _h=hpos, _g=g):
                            if j < Tlo[_g]:
                                return halo[:, _l + j, :]
                            return hahi[:, _h + (j - Tlo[_g]), :]

                        s_ps = psc.tile([P, Tg], F32, tag="s", name=f"s{g}")
                        aggr = pag.tile([P, P], F32, tag="aggr", name=f"ag{g}")
                        w_sb = wrk.tile([P, Tg], F32, tag="w", name=f"w{g}")
                        nck = (Tg + SCORE_BATCH - 1) // SCORE_BATCH
                        if DBG:
                            nck = 0
                        for c in range(nck):
                            tl, th = c * SCORE_BATCH, min((c + 1) * SCORE_BATCH, Tg)
                            z = pz.tile(
                                [P, SCORE_BATCH * P], F32, tag="z",
                                name=f"z{g}_{c}",
                            )
                            for i, t in enumerate(range(tl, th)):
                                zsl = slice(i * P, (i + 1) * P)
                                nc.tensor.matmul(
                                    out=z[:, zsl], lhsT=ha_t(t)[:, D : 2 * D],
                                    rhs=idn_sb[:], start=True, stop=False,
                                )
                                nc.tensor.matmul(
                                    out=z[:, zsl], lhsT=btc[:, tpos + t, :],
                                    rhs=idn_sb[:], start=False, stop=True,
                                )
                            wl = (th - tl) * P
                            r = wrk.tile(
                                [P, SCORE_BATCH * P], BF, tag="r",
                                name=f"r{g}_{c}",
                            )
                            nc.scalar.activation(
                                out=r[:, 0:wl], in_=z[:, 0:wl],
                                func=mybir.ActivationFunctionType.Relu,
                            )
                            for i, t in enumerate(range(tl, th)):
                                nc.tensor.matmul(
                                    out=s_ps[:, t : t + 1],
                                    lhsT=r[:, i * P : (i + 1) * P],
                                    rhs=We2_sb[:], start=True, stop=True,
                                )
                        if not DBG:
                            nc.scalar.activation(
                                out=w_sb[:], in_=s_ps[:, 0:Tg],
                                func=mybir.ActivationFunctionType.Sigmoid,
                                bias=be2_sb[:],
                            )
                        for t in range(Tg if not DBG else 0):
                            m = wrk.tile([P, P], BF, tag="m", name=f"m{g}_{t}")
                            nc.vector.tensor_scalar(
                                out=m[:], in0=iota_sb[:],
                                scalar1=crel_sb[:, t_off + tpos + t : t_off + tpos + t + 1],
                                scalar2=w_sb[:, t : t + 1],
                                op0=mybir.AluOpType.is_equal,
                                op1=mybir.AluOpType.mult,
                            )
                            nc.tensor.matmul(
                                out=aggr[:], lhsT=ha_t(t)[:, 0:D], rhs=m[:],
                                start=(t == 0), stop=(t == Tg - 1),
                            )
                        # ---- phase 3 ----
                        ragg = wrk.tile([P, P], BF, tag="ragg", name=f"rg{g}")
                        if DBG:
                            nc.vector.memset(ragg[:], 0.0)
                        else:
                            nc.any.tensor_copy(out=ragg[:], in_=aggr[:])
                        gp = pg3.tile([P, P], F32, tag="gate", name=f"gp{g}")
                        nc.tensor.matmul(
                            out=gp[:], lhsT=WgT_sb[:], rhs=hT_sb[:, gsl],
                            start=True, stop=False,
                        )
                        nc.tensor.matmul(
                            out=gp[:], lhsT=WgB_sb[:], rhs=ragg[:],
                            start=False, stop=True,
                        )
                        gate = wrk.tile([P, P], BF, tag="gate_sb", name=f"gt{g}")
                        nc.scalar.activation(
                            out=gate[:], in_=gp[:],
                            func=mybir.ActivationFunctionType.Sigmoid,
                            bias=bg_sb[:],
                        )
                        d1 = wrk.tile([P, P], BF, tag="d1", name=f"d1{g}")
                        nc.vector.tensor_tensor(
                            out=d1[:], in0=ragg[:], in1=hT_sb[:, gsl],
                            op=mybir.AluOpType.subtract,
                        )
                        d2 = wrk.tile([P, P], BF, tag="d2", name=f"d2{g}")
                        nc.vector.tensor_mul(out=d2[:], in0=gate[:], in1=d1[:])
                        nc.vector.tensor_add(
                            out=hn_sb[:, gsl], in0=hT_sb[:, gsl], in1=d2[:]
                        )
                        lpos += Tlo[g]
                        hpos += Thi[g]
                        tpos += Tg
                    lo_off += nlo
                    hi_off += nhi
                    t_off += ntc

                # ============== phase 4: final LayerNorm ===============
                for g in range(NG):
                    gsl = slice(g * P, (g + 1) * P)
                    hnp = pg3.tile([P, P], F32, tag="hnp", name=f"hnp{g}")
                    nc.tensor.matmul(
                        out=hnp[:], lhsT=hn_sb[:, gsl], rhs=idn_sb[:],
                        start=True, stop=True,
                    )
                    st3 = wrk.tile([P, 6], F32, tag="st3", name=f"st3{g}")
                    nc.vector.bn_stats(out=st3[:], in_=hnp[:])
                    mv3 = wrk.tile([P, 2], F32, tag="mv3", name=f"mv3{g}")
                    nc.vector.bn_aggr(out=mv3[:], in_=st3[:])
                    sd3 = wrk.tile([P, 1], F32, tag="sd3", name=f"sd3{g}")
                    nc.scalar.activation(
                        out=sd3[:], in_=mv3[:, 1:2],
                        func=mybir.ActivationFunctionType.Sqrt,
                        bias=eps_sb[:],
                    )
                    rstd3 = wrk.tile([P, 1], F32, tag="rstd3", name=f"rs3{g}")
                    nc.vector.reciprocal(out=rstd3[:], in_=sd3[:])
                    t1o = osb.tile([P, D], F32, tag="t1o", name=f"t1o{g}")
                    nc.vector.tensor_scalar(
                        out=t1o[:], in0=hnp[:], scalar1=mv3[:, 0:1],
                        scalar2=rstd3[:], op0=mybir.AluOpType.subtract,
                        op1=mybir.AluOpType.mult,
                    )
                    u1o = osb.tile([P, D], F32, tag="u1o", name=f"u1o{g}")
                    nc.vector.tensor_mul(out=u1o[:], in0=t1o[:], in1=g2_sb[:])
                    o1o = osb.tile([P, D], F32, tag="o1o", name=f"o1o{g}")
                    nc.vector.tensor_add(out=o1o[:], in0=u1o[:], in1=bt2_sb[:])
                    nc.sync.dma_start(out=out[gsl, :], in_=o1o[:])

    nc.compile()
    return nc


# ----------------------------------------------------------------------------
# host-side sharding + launch
# ----------------------------------------------------------------------------

_CACHE = {}


def _wrap16(seq):
    """idx i -> [i%16, i//16], replicated to 128 partitions (8 Q7 cores)."""
    n = len(seq)
    assert n % 16 == 0
    blk = np.asarray(seq, np.int16).reshape(-1, 16).T  # [16, n/16]
    return np.tile(blk, (8, 1))                        # [128, n/16]


def kernel(
    x, edge_index, W1, b1, g1, bt1, We1, be1, We2, be2,
    Wn1, bn1, Wn2, bn2, Wg, bg, g2, bt2, _trace=False,
):
    x = np.asarray(x, dtype=np.float32)
    N = x.shape[0]
    NG = (N + N_CORES * P - 1) // (N_CORES * P)
    NLOC = NG * P
    NPAD = NLOC * N_CORES

    row = np.asarray(edge_index[0], dtype=np.int64)
    col = np.asarray(edge_index[1], dtype=np.int64)
    order = np.argsort(col, kind="stable")
    row_s = row[order].astype(np.int32)
    col_s = col[order].astype(np.int32)

    bounds = np.searchsorted(col_s, np.arange(N_CORES + 1) * NLOC)
    cnt_lo = np.zeros((N_CORES, NG), dtype=np.int64)
    cnt_hi = np.zeros((N_CORES, NG), dtype=np.int64)
    for k in range(N_CORES):
        lo, hi = bounds[k], bounds[k + 1]
        gloc = (col_s[lo:hi] - k * NLOC) // P
        is_hi = row_s[lo:hi] >= SPLIT
        cnt_lo[k] = np.bincount(gloc[~is_hi], minlength=NG)
        cnt_hi[k] = np.bincount(gloc[is_hi], minlength=NG)
    Tlo = [int(math.ceil(cnt_lo[:, g].max() / P)) for g in range(NG)]
    Thi = [int(math.ceil(cnt_hi[:, g].max() / P)) for g in range(NG)]
    for g in range(NG):
        if Tlo[g] + Thi[g] == 0:
            Tlo[g] = 1
    T_list = [lo + hi for lo, hi in zip(Tlo, Thi)]
    NT = sum(T_list)

    key = (N, NG, tuple(Tlo), tuple(Thi))
    if key not in _CACHE:
        _CACHE[key] = _build_program(NG, NLOC, NPAD, Tlo, Thi)
    nc = _CACHE[key]

    bf = lambda a: np.ascontiguousarray(np.asarray(a, np.float32)).astype(BF16)
    f32 = lambda a: np.ascontiguousarray(np.asarray(a, np.float32))
    shared = {
        "W1": f32(W1),
        "b1row": f32(b1).reshape(1, D),
        "ones1f": np.ones((1, D), np.float32),
        "ones1b": np.ones((1, D), BF16),
        "g1b": np.broadcast_to(f32(g1).reshape(1, D), (P, D)).copy(),
        "bt1b": np.broadcast_to(f32(bt1).reshape(1, D), (P, D)).copy(),
        "We1T": bf(We1[:D]),
        "We1B": bf(We1[D:]),
        "be1row": bf(be1).reshape(1, D),
        "We2c": bf(We2).reshape(P, 1),
        "be2c": np.broadcast_to(f32(be2).reshape(1, 1), (P, 1)).copy(),
        "WgT": bf(Wg[:D]),
        "WgB": bf(Wg[D:]),
        "bgc": f32(bg).reshape(P, 1),
        "g2b": np.broadcast_to(f32(g2).reshape(1, D), (P, D)).copy(),
        "bt2b": np.broadcast_to(f32(bt2).reshape(1, D), (P, D)).copy(),
        "idn": np.eye(P, dtype=BF16),
        "iota": np.broadcast_to(
            np.arange(P, dtype=np.float32).reshape(1, P), (P, P)
        ).astype(BF16),
    }

    xp = np.zeros((NPAD, D), np.float32)
    xp[:N] = x

    in_maps = []
    for k in range(N_CORES):
        lo, hi = bounds[k], bounds[k + 1]
        rk = row_s[lo:hi]
        ck = col_s[lo:hi] - k * NLOC
        gk = ck // P
        is_hi = rk >= SPLIT
        seq_lo = []     # row idx (lo), per group padded
        seq_hi = []     # row idx - SPLIT (hi)
        seq_b = []      # col idx, combined tile order
        seq_cr = []     # col - group_base (or -1), combined tile order
        for g in range(NG):
            sel = gk == g
            for half, Tn, base in ((~is_hi & sel, Tlo[g], 0),
                                   (is_hi & sel, Thi[g], SPLIT)):
                n = int(half.sum())
                npad = Tn * P
                r_g = np.zeros(npad, np.int32)
                c_g = np.zeros(npad, np.int32)
                x_g = np.full(npad, -1.0, np.float32)
                r_g[:n] = rk[half] - base
                c_g[:n] = ck[half]
                x_g[:n] = (c_g[:n] - g * P).astype(np.float32)
                (seq_lo if base == 0 else seq_hi).append(r_g)
                seq_b.append(c_g)
                seq_cr.append(x_g)
        cat = lambda s, n: (np.concatenate(s) if s else np.zeros(0, np.int32))
        slo = np.concatenate(seq_lo) if seq_lo else np.zeros(0, np.int32)
        shi = np.concatenate(seq_hi) if seq_hi else np.zeros(0, np.int32)
        sb = np.concatenate(seq_b)
        scr = np.concatenate(seq_cr)
        if len(slo) == 0:
            slo = np.zeros(P, np.int32)
        if len(shi) == 0:
            shi = np.zeros(P, np.int32)
        im = dict(shared)
        im["xT"] = np.ascontiguousarray(xp[k * NLOC : (k + 1) * NLOC].T)
        im["ixlo"] = _wrap16(slo)
        im["ixhi"] = _wrap16(shi)
        im["ixb"] = _wrap16(sb)
        im["colrel"] = np.ascontiguousarray(
            scr.reshape(NT, P).T
        )
        in_maps.append(im)

    if _trace:
        _install_ntff_hook()
    res = run_bass_kernel_spmd(
        nc, in_maps, core_ids=list(range(N_CORES)), trace=_trace
    )
    out = np.concatenate(
        [res.results[k]["out"] for k in range(N_CORES)], axis=0
    )[:N]
    if _trace:
        kernel.last_exec_time_ns = res.exec_time_ns
    return np.ascontiguousarray(out, dtype=np.float32)
